# revision 7
# baseline (speedup 1.0000x reference)
"""YOLOv1 loss kernel for Trainium2 (8 NeuronCores, data-parallel over batch).

Strategy: the loss splits exactly into
    total = sum_{obj cells} (coor_sel + e_sel^2 + 0.5*e_oth^2 + cls)
          + sum_{noobj cells} 0.5*(p4^2 + p9^2)
where obj = (labels[:,4] == 1.0).  Only ~30% of cells are obj.  The host
packer (a permutation + fp16 cast + constant channel scaling) partitions
cells by the obj flag per core:
  - obj region: 32 channels/cell, full IoU/coor/conf/cls pipeline.
  - noobj region: only pred conf channels (2/cell); the whole
    contribution is ONE ACT Square(scale sqrt(.5)) with accum_out.
This cuts DMA from 6.6MB to ~2.2MB/core and elementwise work ~70%.

Channel blocks (cells along columns, 128 partitions; within 4W blocks
the order is [*_box1, *_box2] per axis so x/y pair via W-strided views):
  PQs = 3.5*pred [w1,w2,h1,h2]  (ch 2,7,3,8)   cols  0: 4W
  LSQ = 3.5*lab  [wg,l7,hg,l8]  (ch 2,7,3,8)   cols  4: 8W
  PA  = pred [x1,x2,y1,y2]      (ch 0,5,1,6)   cols  8:12W
  PC  = pred cls                (ch 10..16)    cols 12:19W
  LXY = lab  [xg,l5,yg,l6]      (ch 0,5,1,6)   cols 19:23W
  LC  = lab  cls                (ch 10..16)    cols 23:30W
  PF  = pred [c1,c2]            (ch 4,9)       cols 30:32W
The 3.5 pre-scale makes PQs/LSQ the IoU half-widths directly; the coor
sqrt terms absorb it via the sds ACT scale sqrt(10/7) ((sqrt(3.5p) -
sqrt(3.5l))^2 = 3.5*(sqrt p - sqrt l)^2).  [PA|PC] vs [LXY|LC] are
adjacent so one 11W subtract yields all coor-xy and cls diffs.  The
ground box (xg,yg,wg,hg) doubles as the coor1 target.

Padding cells (to equalize the 8 cores' shapes) use identical pred/label
boxes with conf=1: contribution is only LUT roundoff (~1e-5 each).

IoU: translation invariance drops the grid offsets; with coords x7 the
box is center=x, half=3.5w, and inter/areas share a 1/4 factor that
cancels in inter/union.

ACT tables: sqrt and rsqrt never share a set, but square is in every
set.  Order: warm Sqrt (loads during DMA fill), spl, then a dummy warm
Rsqrt triggers the single switch ~3us in (hidden under DVE box math);
all later squares and the real Rsqrt run from the rsqrt set.
"""

import numpy as np

B = 16384
NCORES = 8
BL = B // NCORES
CELLS = 49
NFLAT = BL * CELLS        # 100352 cells per core
P = 128

SQRT5 = float(np.sqrt(5.0))
SQRTH = float(np.sqrt(0.5))
SDS_SCALE = float(np.sqrt(10.0 / 7.0))

# channel gather orders (index into the 17 channels)
_PRED_WH = [2, 7, 3, 8]
_LAB_WH = [2, 7, 3, 8]
_PRED_XY = [0, 5, 1, 6]
_LAB_XY = [0, 5, 1, 6]
_CLS = [10, 11, 12, 13, 14, 15, 16]

# pad cell: identical boxes (0.5 everywhere), conf 1.0 -> contribution ~0
_PAD = np.zeros(32, np.float16)
_PAD[0:8] = 1.75          # PQs, LSQ (3.5 * 0.5)
_PAD[8:30] = 0.5          # PA, PC, LXY, LC
_PAD[30:32] = 1.0         # PF


def _pack_all(pred, labels):
    """-> (xo (NC,P,32*WO) f16, xn (NC,P,2*WN) f16, WO, WN)"""
    prd = np.ascontiguousarray(
        np.asarray(pred, np.float32).reshape(NCORES, BL, 17, CELLS)
        .transpose(0, 2, 1, 3)).reshape(NCORES, 17, NFLAT)
    lab = np.ascontiguousarray(
        np.asarray(labels, np.float32).reshape(NCORES, BL, 17, CELLS)
        .transpose(0, 2, 1, 3)).reshape(NCORES, 17, NFLAT)
    objf = lab[:, 4, :] == 1.0
    counts = objf.sum(1)
    WO = max(1, -(-int(counts.max()) // P))
    WN = max(1, -(-int(NFLAT - counts.min()) // P))
    NO = P * WO
    NN = P * WN

    xo = np.empty((NCORES, 32, NO), np.float16)
    xn = np.zeros((NCORES, 2, NN), np.float16)
    for i in range(NCORES):
        oi = np.flatnonzero(objf[i])
        ni = np.flatnonzero(~objf[i])
        c = len(oi)
        xo[i, 0:4, :c] = 3.5 * prd[i][_PRED_WH][:, oi]
        xo[i, 4:8, :c] = 3.5 * lab[i][_LAB_WH][:, oi]
        xo[i, 8:12, :c] = prd[i][_PRED_XY][:, oi]
        xo[i, 12:19, :c] = prd[i][_CLS][:, oi]
        xo[i, 19:23, :c] = lab[i][_LAB_XY][:, oi]
        xo[i, 23:30, :c] = lab[i][_CLS][:, oi]
        xo[i, 30:32, :c] = prd[i][[4, 9]][:, oi]
        xo[i, :, c:] = _PAD[:, None]
        xn[i, :, :len(ni)] = prd[i][[4, 9]][:, ni]
    # (NC,32,NO) -> (NC,P,32,WO) -> (NC,P,32*WO); cell k = p*WO + j
    xo = xo.reshape(NCORES, 32, P, WO).transpose(0, 2, 1, 3)
    xo = np.ascontiguousarray(xo).reshape(NCORES, P, 32 * WO)
    xn = xn.reshape(NCORES, 2, P, WN).transpose(0, 2, 1, 3)
    xn = np.ascontiguousarray(xn).reshape(NCORES, P, 2 * WN)
    return xo, xn, WO, WN


def _act_rsqrt(nc, mybir, out, in_):
    """ScalarE Rsqrt via raw InstActivation (bass wrapper bans Rsqrt);
    1/union = rsqrt(union)^2, measured end-to-end error ~1e-5."""
    imm = lambda v: mybir.ImmediateValue(dtype=mybir.dt.float32, value=v)
    eng = nc.scalar
    inst = mybir.InstActivation(
        name=nc.get_next_instruction_name(),
        func=mybir.ActivationFunctionType.Rsqrt,
        ins=[eng.lower_ap(in_), imm(0.0), imm(1.0), imm(0.0)],
        outs=[eng.lower_ap(out)],
    )
    return eng.add_instruction(inst)


def _build_nc(WO, WN):
    import concourse.bass as bass
    import concourse.mybir as mybir
    from concourse.tile import TileContext
    from concourse.alu_op_type import AluOpType as op

    CT = mybir.dt.float16
    F32 = mybir.dt.float32
    SQ = mybir.ActivationFunctionType.Square
    SQRT = mybir.ActivationFunctionType.Sqrt
    W = WO

    nc = bass.Bass()
    xo_in = nc.dram_tensor("xo", [P, 32 * W], CT, kind="ExternalInput")
    xn_in = nc.dram_tensor("xn", [P, 2 * WN], CT, kind="ExternalInput")
    acc_out = nc.dram_tensor("acc", [P, 2], F32, kind="ExternalOutput")

    def v22(ap):   # [P,4W] -> [P,2,2,W] (axis, box, w)
        return ap.rearrange("p (a o w) -> p a o w", a=2, o=2)

    def v21(ap):   # [P,2W] -> [P,2,1,W]
        return ap.rearrange("p (a o w) -> p a o w", a=2, o=1)

    def bco(ap):   # [P,2,1,W] -> [P,2,2,W]
        return ap.broadcast_to([P, 2, 2, W])

    with TileContext(nc) as tc:
        with (
            tc.tile_pool(name="inp", bufs=1) as inpool,
            tc.tile_pool(name="mid", bufs=1) as mid,
            tc.tile_pool(name="accp", bufs=1) as accp,
        ):
            acc = accp.tile([P, 2], F32)
            warm = accp.tile([P, 2], CT)
            nc.vector.memset(warm[:], 1.0)
            wo_ = accp.tile([P, 2], CT)
            # loads the sqrt table set during the DMA fill
            nc.scalar.activation(out=wo_[:], in_=warm[:], func=SQRT)

            xot = inpool.tile([P, 32 * W], CT)
            nc.sync.dma_start(out=xot[:, 0:4 * W], in_=xo_in[:, 0:4 * W])
            nc.sync.dma_start(out=xot[:, 8 * W:12 * W], in_=xo_in[:, 8 * W:12 * W])
            nc.sync.dma_start(out=xot[:, 4 * W:8 * W], in_=xo_in[:, 4 * W:8 * W])
            nc.sync.dma_start(out=xot[:, 19 * W:23 * W],
                              in_=xo_in[:, 19 * W:23 * W])
            xnt = inpool.tile([P, 2 * WN], CT)
            nc.sync.dma_start(out=xnt[:], in_=xn_in[:])
            nc.sync.dma_start(out=xot[:, 12 * W:19 * W],
                              in_=xo_in[:, 12 * W:19 * W])
            nc.sync.dma_start(out=xot[:, 23 * W:32 * W],
                              in_=xo_in[:, 23 * W:32 * W])

            PQs = xot[:, 0:4 * W]            # 3.5*[w1,w2,h1,h2]
            LSQ = xot[:, 4 * W:8 * W]        # 3.5*[wg,l7,hg,l8]
            PA = xot[:, 8 * W:12 * W]        # [x1,x2,y1,y2]
            PACM = xot[:, 8 * W:19 * W]      # [PA|PC] for the 11W diff
            LXY = xot[:, 19 * W:23 * W]      # [xg,l5,yg,l6]
            LXCM = xot[:, 19 * W:30 * W]     # [LXY|LC]
            PF = xot[:, 30 * W:32 * W]       # [c1,c2]
            LSQg = v22(LSQ)[:, :, 0:1]       # [P,2,1,W] = 3.5*[wg,hg]
            LXYg = v22(LXY)[:, :, 0:1]       # [P,2,1,W] = [xg,yg]

            # --- ACT stream part 1 (sqrt set, then hidden switch) ---
            spl = mid.tile([P, 8 * W], CT)
            nc.scalar.activation(out=spl[:], in_=xot[:, 0:8 * W], func=SQRT)
            # dummy rsqrt: pulls the 1.28us table switch early, under DVE
            # work; reads spl so the scheduler keeps it after the real Sqrt
            _act_rsqrt(nc, mybir, wo_[:], spl[:, 0:2])

            # --- DVE box math (queue order ~= execution order) ---
            arp = mid.tile([P, 2 * W], CT)
            nc.vector.tensor_tensor(out=arp[:], in0=PQs[:, 0:2 * W],
                                    in1=PQs[:, 2 * W:4 * W], op=op.mult)
            x1p = mid.tile([P, 4 * W], CT)
            nc.vector.tensor_tensor(out=x1p[:], in0=PA, in1=PQs, op=op.subtract)
            x2p = mid.tile([P, 4 * W], CT)
            nc.vector.tensor_tensor(out=x2p[:], in0=PA, in1=PQs, op=op.add)
            arg = mid.tile([P, W], CT)
            nc.vector.tensor_tensor(out=arg[:], in0=LSQ[:, 0:W],
                                    in1=LSQ[:, 2 * W:3 * W], op=op.mult)
            x1g = mid.tile([P, 2 * W], CT)
            nc.vector.tensor_tensor(out=v21(x1g[:]), in0=LXYg, in1=LSQg,
                                    op=op.subtract)
            x2g = mid.tile([P, 2 * W], CT)
            nc.vector.tensor_tensor(out=v21(x2g[:]), in0=LXYg, in1=LSQg,
                                    op=op.add)
            imax = mid.tile([P, 4 * W], CT)
            nc.vector.tensor_tensor(out=v22(imax[:]), in0=v22(x1p[:]),
                                    in1=bco(v21(x1g[:])), op=op.max)
            imin = mid.tile([P, 4 * W], CT)
            nc.vector.tensor_tensor(out=v22(imin[:]), in0=v22(x2p[:]),
                                    in1=bco(v21(x2g[:])), op=op.min)
            dd = mid.tile([P, 4 * W], CT)
            nc.vector.tensor_tensor(out=dd[:], in0=imin[:], in1=imax[:],
                                    op=op.subtract)
            dr = mid.tile([P, 4 * W], CT)
            nc.vector.tensor_scalar(out=dr[:], in0=dd[:], scalar1=0.0,
                                    scalar2=0.5, op0=op.max, op1=op.mult)
            inter = mid.tile([P, 2 * W], CT)
            nc.vector.tensor_tensor(out=inter[:], in0=dr[:, 0:2 * W],
                                    in1=dr[:, 2 * W:4 * W], op=op.mult)
            uu = mid.tile([P, 2 * W], CT)
            nc.vector.tensor_tensor(
                out=uu[:].rearrange("p (o w) -> p o w", o=2),
                in0=arp[:].rearrange("p (o w) -> p o w", o=2),
                in1=arg[:].rearrange("p (o w) -> p o w", o=1)
                .broadcast_to([P, 2, W]), op=op.add)
            un = mid.tile([P, 2 * W], CT)
            nc.vector.tensor_tensor(out=un[:], in0=uu[:], in1=inter[:],
                                    op=op.subtract)
            sd = mid.tile([P, 4 * W], CT)
            nc.vector.tensor_tensor(out=sd[:], in0=spl[:, 0:4 * W],
                                    in1=spl[:, 4 * W:8 * W], op=op.subtract)
            diffac = mid.tile([P, 11 * W], CT)
            nc.vector.tensor_tensor(out=diffac[:], in0=PACM, in1=LXCM,
                                    op=op.subtract)

            # --- ACT stream part 2 (rsqrt set; squares are in every set) ---
            ppsn = mid.tile([P, 2 * WN], CT)
            nc.scalar.activation(out=ppsn[:], in_=xnt[:], func=SQ, scale=SQRTH,
                                 accum_out=acc[:, 1:2])
            rc = mid.tile([P, 2 * W], CT)
            _act_rsqrt(nc, mybir, rc[:], un[:])
            rc2 = mid.tile([P, 2 * W], CT)
            nc.scalar.activation(out=rc2[:], in_=rc[:], func=SQ)
            dsqa = mid.tile([P, 4 * W], CT)
            nc.scalar.activation(out=dsqa[:], in_=diffac[:, 0:4 * W], func=SQ,
                                 scale=SQRT5)
            sds = mid.tile([P, 4 * W], CT)
            nc.scalar.activation(out=sds[:], in_=sd[:], func=SQ, scale=SDS_SCALE)

            iou = mid.tile([P, 2 * W], CT)
            nc.vector.tensor_tensor(out=iou[:], in0=inter[:], in1=rc2[:],
                                    op=op.mult)
            u1c = mid.tile([P, W], CT)
            nc.vector.tensor_tensor(out=u1c[:], in0=iou[:, 0:W],
                                    in1=iou[:, W:2 * W], op=op.is_ge)
            e = mid.tile([P, 2 * W], CT)
            nc.vector.tensor_tensor(out=e[:], in0=PF, in1=iou[:], op=op.subtract)
            es = mid.tile([P, 2 * W], CT)
            nc.scalar.activation(out=es[:], in_=e[:], func=SQ, scale=SQRTH)

            dsqc = mid.tile([P, 7 * W], CT)
            nc.vector.tensor_tensor(out=dsqc[:], in0=diffac[:, 4 * W:11 * W],
                                    in1=diffac[:, 4 * W:11 * W], op=op.mult)
            c1 = mid.tile([P, 3 * W], CT)
            nc.vector.tensor_tensor(out=c1[:], in0=dsqc[:, 0:3 * W],
                                    in1=dsqc[:, 3 * W:6 * W], op=op.add)
            c2 = mid.tile([P, W], CT)
            nc.vector.tensor_tensor(out=c2[:], in0=c1[:, 0:W],
                                    in1=c1[:, W:2 * W], op=op.add)
            c3 = mid.tile([P, W], CT)
            nc.vector.tensor_tensor(out=c3[:], in0=c2[:], in1=c1[:, 2 * W:3 * W],
                                    op=op.add)
            clsf = mid.tile([P, W], CT)
            nc.vector.tensor_tensor(out=clsf[:], in0=c3[:],
                                    in1=dsqc[:, 6 * W:7 * W], op=op.add)

            # --- merge tail ---
            tq = mid.tile([P, 4 * W], CT)
            nc.vector.tensor_tensor(out=tq[:], in0=dsqa[:], in1=sds[:], op=op.add)
            coorp = mid.tile([P, 2 * W], CT)
            nc.vector.tensor_tensor(out=coorp[:], in0=tq[:, 0:2 * W],
                                    in1=tq[:, 2 * W:4 * W], op=op.add)
            esum = mid.tile([P, W], CT)
            nc.vector.tensor_tensor(out=esum[:], in0=es[:, 0:W],
                                    in1=es[:, W:2 * W], op=op.add)
            aq = mid.tile([P, 2 * W], CT)
            nc.vector.tensor_tensor(out=aq[:], in0=coorp[:], in1=es[:], op=op.add)
            da = mid.tile([P, W], CT)
            nc.vector.tensor_tensor(out=da[:], in0=aq[:, 0:W],
                                    in1=aq[:, W:2 * W], op=op.subtract)
            sa = mid.tile([P, W], CT)
            nc.vector.tensor_tensor(out=sa[:], in0=u1c[:], in1=da[:], op=op.mult)
            sel = mid.tile([P, W], CT)
            nc.vector.tensor_tensor(out=sel[:], in0=sa[:], in1=aq[:, W:2 * W],
                                    op=op.add)
            o2 = mid.tile([P, W], CT)
            nc.vector.tensor_tensor(out=o2[:], in0=sel[:], in1=esum[:], op=op.add)
            o3 = mid.tile([P, W], CT)
            nc.vector.tensor_tensor(out=o3[:], in0=o2[:], in1=clsf[:], op=op.add)
            nc.vector.tensor_reduce(out=acc[:, 0:1], in_=o3[:],
                                    axis=mybir.AxisListType.X, op=op.add)

            nc.sync.dma_start(out=acc_out[:], in_=acc[:])

    _split_multiwaits(nc, mybir)
    return nc


def _split_multiwaits(nc, mybir, max_waits=1):
    """This walrus build rejects instructions carrying more than one sem
    wait; hoist extra waits onto same-engine Drain instructions inserted
    immediately before the offender (semantically identical stall point)."""
    ctr = [0]
    for bb in nc.main_func.blocks:
        insts = bb.instructions
        out = []
        for ins in insts:
            si = ins.sync_info
            if si is not None and si.on_wait and len(si.on_wait) > max_waits:
                waits = list(si.on_wait)
                extra, keep = waits[:-max_waits], waits[-max_waits:]
                for k in range(0, len(extra), max_waits):
                    d = mybir.InstDrain(name=f"I-mw{ctr[0]}", ins=[], outs=[])
                    ctr[0] += 1
                    d.engine = ins.engine
                    d.sync_info = mybir.SyncInfo(on_wait=extra[k:k + max_waits],
                                                 on_update=[])
                    nc.register_instruction(d)
                    out.append(d)
                ins.sync_info = mybir.SyncInfo(on_wait=keep,
                                               on_update=list(si.on_update or []))
            out.append(ins)
        bb.instructions = out


_CACHED = {}


def kernel(pred, labels):
    from concourse.bass_utils import run_bass_kernel_spmd

    xo, xn, WO, WN = _pack_all(pred, labels)
    key = (WO, WN)
    if key not in _CACHED:
        _CACHED.clear()
        _CACHED[key] = _build_nc(WO, WN)
    nc = _CACHED[key]

    in_maps = [{"xo": xo[i], "xn": xn[i]} for i in range(NCORES)]
    res = run_bass_kernel_spmd(nc, in_maps, core_ids=list(range(NCORES)))
    total = np.float64(0.0)
    for i in range(NCORES):
        total += res.results[i]["acc"].astype(np.float64).sum()
    return np.asarray(total / B, dtype=np.float32)
